# revision 14
# baseline (speedup 1.0000x reference)
"""Trainium2 Bass/Tile kernel for a BitNet-style fused observation block.

Computation (per reference):
  combined = concat([z_byte, z_addr, z_evt, z_map, z_sum], -1)        # [B, 2048]
  g = TL2(gelu(TL1(combined)))        (TL = ternary-quantized linear)
  t = TL2'(gelu(TL1'(combined)))
  fused = sigmoid(1.2*g) * t
  fused = LN1(fused)
  mlp   = gelu(fused @ mlp_w1.T + b1) @ mlp_w2.T + b2
  out   = LN2(fused + mlp)                                            # [B, 2048]

Strategy (v2): data-parallel over batch (1024 rows/core, 8 cores), activations
transposed [feature, batch] so features are the contraction dim.

Key speedups over v1:
  * Ternary matmuls run in fp8 DoubleRow mode (K=256 per instruction,
    ~105ns/MM vs 213ns bf16 => 2x).  Ternary weights are exact in fp8; the
    activations are sent as a split pair x = x8 + xr8 (both fp8e4m3), so the
    pair of half-chains reproduces bf16-level accuracy while running at fp8
    DoubleRow speed.
  * LN1 is folded into the mlp_w1 matmul: W1 @ LN1(f) = rs*(W1g @ f)
    - m*rs*c1 + c3 with W1g = mlp_w1*diag(ln1_g), c1 = mlp_w1@ln1_g,
    c3 = mlp_w1@ln1_b + mlp_b1 (host-precomputed).  Removes the LN1
    normalize barrier between the gate/transform part and the MLP.
  * sigmoid(1.2g) computed as 0.5*tanh(0.6g)+0.5 (tanh lives in the same
    ACT table set as gelu -> fewer ~2.7us table switches).
  * LN2 normalize+store of half h is emitted after half h+1's first matmul
    phase, so it overlaps the tensor engine instead of serializing.

LayerNorm reductions over features (= partitions) use the TensorEngine with
an all-ones [128,128] stationary operand (column sums broadcast across
partitions).  Each core processes its 1024 batch columns in two sequential
half-batches of 512 (weights streamed twice).
"""

import os

import numpy as np
import ml_dtypes

BF16 = ml_dtypes.bfloat16

# Problem dims (hardcoded per the harness contract).
B_TOTAL = 8192
N_CORES = 8
B_CORE = B_TOTAL // N_CORES  # 1024
D = 2048                     # IN == F == 2048
H = 4096                     # MLP hidden
EPS = 1e-5
Z_ORDER = ("z_byte", "z_addr", "z_evt", "z_map", "z_sum")


# ---------------------------------------------------------------------------
# Host-side packing helpers
# ---------------------------------------------------------------------------

def _absmean_scale(w):
    """BitNet absmean scale, matching jnp.mean(jnp.abs(w)) in f32."""
    try:
        import jax
        import jax.numpy as jnp

        cpu = jax.devices("cpu")[0]
        with jax.default_device(cpu):
            s = jnp.mean(jnp.abs(jnp.asarray(w, dtype=jnp.float32)))
            return float(s)
    except Exception:
        return float(np.mean(np.abs(w), dtype=np.float32))


def _ternary(w):
    """Return ({-1,0,1} float32 matrix, scale) for BitNet quantization."""
    w = np.asarray(w, dtype=np.float32)
    s = _absmean_scale(w)
    t = np.round(np.clip(w / np.float32(s + 1e-5), -1.0, 1.0)).astype(np.float32)
    return t, s


def _pack_lhsT(mat, dtype=BF16):
    """Pack [F_out, K] weight into lhsT DRAM layout [ft, ki, ko, f] where
    element = mat[ft*128 + f, ko*128 + ki]."""
    f_out, k = mat.shape
    arr = mat.reshape(f_out // 128, 128, k // 128, 128)  # [ft, f, ko, ki]
    return np.ascontiguousarray(arr.transpose(0, 3, 2, 1)).astype(dtype)


def _pack_lhsT_swi(mat, dtype):
    """Pack [F_out, K] into DoubleRowSwInterleave lhsT layout
    [ft, ki, kp, c, i]: element = mat[ft*128 + (127-c), (2kp+i)*128 + ki]."""
    f_out, k = mat.shape
    arr = mat.reshape(f_out // 128, 128, k // 128, 128)  # [ft, m, ko, ki]
    a = arr[:, ::-1, :, :]                               # [ft, c, ko, ki]
    a = a.reshape(f_out // 128, 128, k // 256, 2, 128)   # [ft, c, kp, i, ki]
    return np.ascontiguousarray(a.transpose(0, 4, 2, 1, 3)).astype(dtype)


def _pack_vec(v):
    """Pack per-feature vector [F] into [128, F//128] (partition, f-tile)."""
    v = np.asarray(v, dtype=np.float32)
    return np.ascontiguousarray(v.reshape(-1, 128).T.astype(np.float32))


# ---------------------------------------------------------------------------
# Device program
# ---------------------------------------------------------------------------

def _build(nc, scales, d=D, h=H, b_core=B_CORE, bb=512, reps=1, tune=None):
    """Emit the per-core Tile program. scales = (sg1, st1, sg2, st2)."""
    import contextlib
    from contextlib import ExitStack

    import concourse.mybir as mybir
    import concourse.tile as tile

    f32 = mybir.dt.float32
    bf16 = mybir.dt.bfloat16
    fp8 = mybir.dt.float8e4
    AF = mybir.ActivationFunctionType
    OP = mybir.AluOpType
    DR = mybir.MatmulPerfMode.DoubleRow

    tune = dict(tune or {})
    w_bufs = tune.get("w_bufs", 3)
    mm_bufs = tune.get("mm_bufs", 4)
    mm6 = tune.get("mm6", 0)
    if mm6:
        mm_bufs = 6
    o_bufs = tune.get("o_bufs", 1)
    sp_bufs = tune.get("sp_bufs", 3)
    interleave = tune.get("interleave", 1)
    mlp_dr = tune.get("mlp_dr", 0)

    sg1, st1, sg2, st2 = scales
    kt = d // 128          # 16 feature tiles of model dim
    ht = h // 128          # 32 feature tiles of mlp hidden
    kp = kt // 2           # 8 DoubleRow k-pairs
    n_half = b_core // bb  # sequential half-batches

    # --- DRAM I/O -----------------------------------------------------------
    x8_d = nc.dram_tensor("x8", [kt, 128, b_core], fp8, kind="ExternalInput")
    xr8_d = nc.dram_tensor("xr8", [kt, 128, b_core], fp8, kind="ExternalInput")
    swi = tune.get("swi", 0)
    if swi:
        wg1_d = nc.dram_tensor("wg1s", [kt, 128, kt // 2, 128, 2], fp8, kind="ExternalInput")
        wt1_d = nc.dram_tensor("wt1s", [kt, 128, kt // 2, 128, 2], fp8, kind="ExternalInput")
        wg2_d = nc.dram_tensor("wg2s", [kt, 128, kt // 2, 128, 2], fp8, kind="ExternalInput")
        wt2_d = nc.dram_tensor("wt2s", [kt, 128, kt // 2, 128, 2], fp8, kind="ExternalInput")
    else:
        wg1_d = nc.dram_tensor("wg1", [kt, 128, kt, 128], fp8, kind="ExternalInput")
        wt1_d = nc.dram_tensor("wt1", [kt, 128, kt, 128], fp8, kind="ExternalInput")
        wg2_d = nc.dram_tensor("wg2", [kt, 128, kt, 128], fp8, kind="ExternalInput")
        wt2_d = nc.dram_tensor("wt2", [kt, 128, kt, 128], fp8, kind="ExternalInput")
    if mlp_dr:
        wm1a_d = nc.dram_tensor("wm1a", [ht, 128, kt, 128], fp8, kind="ExternalInput")
        wm1r_d = nc.dram_tensor("wm1r", [ht, 128, kt, 128], fp8, kind="ExternalInput")
        wm2a_d = nc.dram_tensor("wm2a", [kt, 128, ht, 128], fp8, kind="ExternalInput")
        wm2r_d = nc.dram_tensor("wm2r", [kt, 128, ht, 128], fp8, kind="ExternalInput")
        negc164_d = nc.dram_tensor("negc164", [128, ht], f32, kind="ExternalInput")
        wm1_d = wm2_d = None
    else:
        wm1_d = nc.dram_tensor("wm1", [ht, 128, kt, 128], bf16, kind="ExternalInput")
        wm2_d = nc.dram_tensor("wm2", [kt, 128, ht, 128], bf16, kind="ExternalInput")
    bg1_d = nc.dram_tensor("bg1", [128, kt], f32, kind="ExternalInput")
    bt1_d = nc.dram_tensor("bt1", [128, kt], f32, kind="ExternalInput")
    btanh_d = nc.dram_tensor("btanh", [128, kt], f32, kind="ExternalInput")
    bthalf_d = nc.dram_tensor("bthalf", [128, kt], f32, kind="ExternalInput")
    negc1_d = nc.dram_tensor("negc1", [128, ht], f32, kind="ExternalInput")
    c3_d = nc.dram_tensor("c3", [128, ht], f32, kind="ExternalInput")
    bm2b1_d = nc.dram_tensor("bm2b1", [128, kt], f32, kind="ExternalInput")
    g1s_d = nc.dram_tensor("g1s", [128, kt], f32, kind="ExternalInput")
    gln2_d = nc.dram_tensor("gln2", [128, kt], f32, kind="ExternalInput")
    bln2_d = nc.dram_tensor("bln2", [128, kt], f32, kind="ExternalInput")
    out_d = nc.dram_tensor("outT", [kt, 128, b_core], f32, kind="ExternalOutput")

    x8_ap = x8_d.ap().rearrange("ko ki b -> ki ko b")
    xr8_ap = xr8_d.ap().rearrange("ko ki b -> ki ko b")
    wdict = dict(wg1=wg1_d, wt1=wt1_d, wg2=wg2_d, wt2=wt2_d)
    if mlp_dr:
        wdict.update(wm1a=wm1a_d, wm1r=wm1r_d, wm2a=wm2a_d, wm2r=wm2r_d)
    else:
        wdict.update(wm1=wm1_d, wm2=wm2_d)
    w_aps = {k: v.ap() for k, v in wdict.items()}
    out_ap = out_d.ap()

    with tile.TileContext(nc) as tc, ExitStack() as ctx:
        consts = ctx.enter_context(tc.tile_pool(name="consts", bufs=1))
        big = ctx.enter_context(tc.tile_pool(name="big", bufs=1))
        wpool = ctx.enter_context(tc.tile_pool(name="wpool", bufs=w_bufs))
        spool = ctx.enter_context(tc.tile_pool(name="spool", bufs=2))
        opool = ctx.enter_context(tc.tile_pool(name="opool", bufs=o_bufs))
        mm_ps = ctx.enter_context(tc.tile_pool(name="mm_ps", bufs=mm_bufs, space="PSUM"))
        st_ps = ctx.enter_context(tc.tile_pool(name="st_ps", bufs=1, space="PSUM"))

        # Constants
        ones16 = consts.tile([128, 128], bf16, name="ones16")
        nc.vector.memset(ones16, 1.0)
        eps_t = consts.tile([128, 1], f32, name="eps_t")
        nc.vector.memset(eps_t, EPS)

        def load_vec(dram, n, name):
            t = consts.tile([128, n], f32, name=name)
            nc.sync.dma_start(t, dram.ap())
            return t

        no_wdma = tune.get("no_wdma", 0)
        simple_writers = tune.get("simple_writers", 0)
        dve_writers = tune.get("dve_writers", 1)
        fake_w = []
        if no_wdma:
            for i in range(4):
                f8t = consts.tile([128, kt, 128], fp8, name=f"fw8_{i}")
                nc.vector.memset(f8t, 0.5)
                f16t = consts.tile([128, ht, 128], bf16, name=f"fw16_{i}")
                nc.vector.memset(f16t, 0.5)
                fake_w.append((f8t, f16t))

        dma2 = tune.get("dma2", 0)
        dma_ctr = [0]

        def wtile(kind, n_k, dtype, tag, name, ap):
            if no_wdma:
                t = fake_w[hash(name) % 4][0 if dtype == fp8 else 1]
                return t[:, 0:n_k, :] if n_k != (kt if dtype == fp8 else ht) else t
            if swi and tag == "W8":
                t = wpool.tile([128, n_k // 2, 128, 2], dtype, tag=tag, name=name)
            else:
                t = wpool.tile([128, n_k, 128], dtype, tag=tag, name=name)
            dma_ctr[0] += 1
            if dma2 and dma_ctr[0] % 2:
                nc.scalar.dma_start(t, ap)
            else:
                nc.sync.dma_start(t, ap)
            return t

        bg1 = load_vec(bg1_d, kt, "bg1")
        bt1 = load_vec(bt1_d, kt, "bt1")
        btanh = load_vec(btanh_d, kt, "btanh")
        bthalf = load_vec(bthalf_d, kt, "bthalf")
        negc1 = (load_vec(negc164_d, ht, "negc164") if mlp_dr
                 else load_vec(negc1_d, ht, "negc1"))
        c3 = load_vec(c3_d, ht, "c3")
        bm2b1 = load_vec(bm2b1_d, kt, "bm2b1")
        g1s = load_vec(g1s_d, kt, "g1s")
        gln2 = load_vec(gln2_d, kt, "gln2")
        bln2 = load_vec(bln2_d, kt, "bln2")

        SWI = mybir.MatmulPerfMode.DoubleRowSwInterleave

        def dr_chain(psum, w_tile, rhs8, rhsr8):
            """16 DoubleRow matmuls accumulating W @ (x8 + xr8), K=2048."""
            for j, rhs in ((0, rhs8), (1, rhsr8)):
                for ko in range(kp):
                    if swi:
                        nc.tensor.matmul(
                            psum, w_tile[:, ko],
                            rhs[:, 2 * ko:2 * ko + 2, :],
                            start=(j == 0 and ko == 0),
                            stop=(j == 1 and ko == kp - 1),
                            perf_mode=SWI,
                        )
                    else:
                        nc.tensor.matmul(
                            psum, w_tile[:, 2 * ko:2 * ko + 2, :],
                            rhs[:, 2 * ko:2 * ko + 2, :],
                            start=(j == 0 and ko == 0),
                            stop=(j == 1 and ko == kp - 1),
                            perf_mode=DR,
                        )

        def mm_chain(psum, w_tile, rhs_big, n_k):
            for ko in range(n_k):
                nc.tensor.matmul(
                    psum, w_tile[:, ko, :], rhs_big[:, ko, :],
                    start=(ko == 0), stop=(ko == n_k - 1),
                )

        def dr_chain3(psum, wa, wr, parts8, partsr8):
            """DR chains: Wa@(a8 + r8) + Wr@a8 (scaled fp8 split weights)."""
            segs = [(wa, parts8), (wa, partsr8), (wr, parts8)]
            np_ = len(parts8)
            for si, (w, parts) in enumerate(segs):
                for pi, part in enumerate(parts):
                    for ko in range(kp):
                        nc.tensor.matmul(
                            psum,
                            w[:, pi * kt + 2 * ko:pi * kt + 2 * ko + 2, :],
                            part[:, 2 * ko:2 * ko + 2, :],
                            start=(si == 0 and pi == 0 and ko == 0),
                            stop=(si == 2 and pi == np_ - 1 and ko == kp - 1),
                            perf_mode=DR,
                        )

        # Per-half state (tiles are allocated lazily per half; tags+bufs=1
        # make half h+1 reuse half h's buffers with auto dependencies).
        state = {}

        def emit_A(half):
            b0 = half * bb
            hb = f"h{half}"
            xs8 = big.tile([128, kt, bb], fp8, tag="X8", name=f"xs8_{hb}")
            xsr8 = big.tile([128, kt, bb], fp8, tag="XR8", name=f"xsr8_{hb}")
            nc.sync.dma_start(xs8, x8_ap[:, :, b0:b0 + bb])
            nc.sync.dma_start(xsr8, xr8_ap[:, :, b0:b0 + bb])
            y8g = big.tile([128, kt, bb], fp8, tag="Y8g", name=f"y8g_{hb}")
            yr8g = big.tile([128, kt, bb], fp8, tag="YR8g", name=f"yr8g_{hb}")
            y8t = big.tile([128, kt, bb], fp8, tag="Y8t", name=f"y8t_{hb}")
            yr8t = big.tile([128, kt, bb], fp8, tag="YR8t", name=f"yr8t_{hb}")
            for ft in range(kt):
                for wkey, y8, yr8, bias, scale in (
                    ("wg1", y8g, yr8g, bg1, sg1), ("wt1", y8t, yr8t, bt1, st1)):
                    wt = wtile(wkey, kt, fp8, "W8",
                               f"w_{wkey}_{hb}_{ft}", w_aps[wkey][ft])
                    ps = mm_ps.tile([128, bb], f32, tag="mm",
                                    name=f"psA_{wkey}_{hb}_{ft}")
                    dr_chain(ps, wt, xs8, xsr8)
                    ytmp = spool.tile([128, bb], bf16, tag="ytmp", bufs=sp_bufs,
                                      name=f"ytmp_{wkey}_{hb}_{ft}")
                    if simple_writers:
                        nc.scalar.activation(y8[:, ft, :], ps, AF.Gelu,
                                             bias=bias[:, ft:ft + 1], scale=scale)
                        nc.vector.tensor_copy(yr8[:, ft, :], y8[:, ft, :])
                    elif dve_writers:
                        nc.scalar.activation(ytmp, ps, AF.Gelu,
                                             bias=bias[:, ft:ft + 1], scale=scale)
                        nc.vector.tensor_copy(y8[:, ft, :], ytmp)
                        nc.vector.tensor_sub(yr8[:, ft, :], ytmp, y8[:, ft, :])
                    else:
                        nc.scalar.activation(ytmp, ps, AF.Gelu,
                                             bias=bias[:, ft:ft + 1], scale=scale)
                        nc.gpsimd.tensor_copy(y8[:, ft, :], ytmp)
                        nc.vector.tensor_sub(yr8[:, ft, :], ytmp, y8[:, ft, :])
            state[half] = dict(y8g=y8g, yr8g=yr8g, y8t=y8t, yr8t=yr8t)

        def stats(src_big, tag, hb):
            """Column mean + rsqrt(var+eps) of [128, kt, bb] bf16 via TensorE.
            Returns (m, rs) f32 [128, bb] tiles (broadcast across partitions)."""
            stag = "" if mm6 else tag
            s1 = st_ps.tile([128, bb], f32, tag=f"s1{stag}", name=f"s1_{tag}_{hb}")
            s2 = st_ps.tile([128, bb], f32, tag=f"s2{stag}", name=f"s2_{tag}_{hb}")
            for j in range(kt):
                xsq = spool.tile([128, bb], bf16, tag="xsq", bufs=sp_bufs,
                                 name=f"xsq_{tag}_{hb}_{j}")
                nc.scalar.activation(xsq, src_big[:, j, :], AF.Square)
                nc.tensor.matmul(s1, ones16, src_big[:, j, :],
                                 start=(j == 0), stop=(j == kt - 1))
                nc.tensor.matmul(s2, ones16, xsq,
                                 start=(j == 0), stop=(j == kt - 1))
            m = spool.tile([128, bb], f32, tag=f"m{tag}", bufs=1,
                           name=f"m_{tag}_{hb}")
            nc.vector.tensor_scalar_mul(m, s1, 1.0 / d)
            msq = spool.tile([128, bb], f32, tag="msq", bufs=2,
                             name=f"msq_{tag}_{hb}")
            nc.vector.tensor_mul(msq, m, m)
            var = spool.tile([128, bb], f32, tag="var", bufs=2,
                             name=f"var_{tag}_{hb}")
            nc.vector.scalar_tensor_tensor(var, s2, 1.0 / d, msq,
                                           OP.mult, OP.subtract)
            sd = spool.tile([128, bb], f32, tag="sd", bufs=2,
                            name=f"sd_{tag}_{hb}")
            nc.scalar.activation(sd, var, AF.Sqrt, bias=eps_t[:, 0:1], scale=1.0)
            rs = spool.tile([128, bb], f32, tag=f"rs{tag}", bufs=1,
                            name=f"rs_{tag}_{hb}")
            nc.vector.reciprocal(rs, sd)
            return m, rs

        def emit_B(half):
            hb = f"h{half}"
            st = state[half]
            fr16 = big.tile([128, kt, bb], bf16, tag="F16", name=f"fr16_{hb}")
            if mlp_dr:
                f8 = big.tile([128, kt, bb], fp8, tag="F8", name=f"f8_{hb}")
                fr8 = big.tile([128, kt, bb], fp8, tag="FR8", name=f"fr8_{hb}")
            for ft in range(kt):
                wg = wtile("wg2", kt, fp8, "W8", f"w_wg2_{hb}_{ft}",
                           w_aps["wg2"][ft])
                psg = mm_ps.tile([128, bb], f32, tag="mm", name=f"psBg_{hb}_{ft}")
                dr_chain(psg, wg, st["y8g"], st["yr8g"])
                wt2 = wtile("wt2", kt, fp8, "W8", f"w_wt2_{hb}_{ft}",
                            w_aps["wt2"][ft])
                pst = mm_ps.tile([128, bb], f32, tag="mm", name=f"psBt_{hb}_{ft}")
                dr_chain(pst, wt2, st["y8t"], st["yr8t"])
                tanhv = spool.tile([128, bb], bf16, tag="tanhv", bufs=sp_bufs,
                                   name=f"tanhv_{hb}_{ft}")
                nc.scalar.activation(tanhv, psg, AF.Tanh,
                                     bias=btanh[:, ft:ft + 1], scale=0.6 * sg2)
                tth = spool.tile([128, bb], bf16, tag="tth", bufs=sp_bufs,
                                 name=f"tth_{hb}_{ft}")
                nc.scalar.activation(tth, pst, AF.Identity,
                                     bias=bthalf[:, ft:ft + 1], scale=0.5 * st2)
                # fused = sigmoid(1.2g)*t = (tanh(0.6g)+1) * (t/2)
                nc.vector.scalar_tensor_tensor(
                    fr16[:, ft, :], tanhv, 1.0, tth, OP.add, OP.mult)
                if mlp_dr:
                    nc.gpsimd.tensor_copy(f8[:, ft, :], fr16[:, ft, :])
                    nc.vector.tensor_sub(fr8[:, ft, :], fr16[:, ft, :],
                                         f8[:, ft, :])
            state[half]["fr16"] = fr16
            if mlp_dr:
                state[half]["f8"] = f8
                state[half]["fr8"] = fr8
            m, rs = stats(fr16, "a", hb)
            state[half]["m1"] = m
            state[half]["rs1"] = rs

        def emit_D(half):
            hb = f"h{half}"
            st = state[half]
            fr16, m1, rs1 = st["fr16"], st["m1"], st["rs1"]
            if mlp_dr:
                h8l = big.tile([128, kt, bb], fp8, tag="Y8g", name=f"h8l_{hb}")
                h8h = big.tile([128, kt, bb], fp8, tag="Y8t", name=f"h8h_{hb}")
                hr8l = big.tile([128, kt, bb], fp8, tag="YR8g", name=f"hr8l_{hb}")
                hr8h = big.tile([128, kt, bb], fp8, tag="YR8t", name=f"hr8h_{hb}")
                f8, fr8 = st["f8"], st["fr8"]
                for ft in range(ht):
                    wa = wtile("wm1a", kt, fp8, "W8", f"w_wm1a_{hb}_{ft}",
                               w_aps["wm1a"][ft])
                    wr = wtile("wm1r", kt, fp8, "W8", f"w_wm1r_{hb}_{ft}",
                               w_aps["wm1r"][ft])
                    ps = mm_ps.tile([128, bb], f32, tag="mm",
                                    name=f"psD_{hb}_{ft}")
                    dr_chain3(ps, wa, wr, [f8], [fr8])
                    u1 = spool.tile([128, bb], f32, tag="u1", bufs=sp_bufs,
                                    name=f"u1_{hb}_{ft}")
                    nc.vector.scalar_tensor_tensor(
                        u1, m1, negc1[:, ft:ft + 1], ps, OP.mult, OP.add)
                    u2 = spool.tile([128, bb], bf16, tag="u2", bufs=sp_bufs,
                                    name=f"u2_{hb}_{ft}")
                    nc.gpsimd.tensor_mul(u2, u1, rs1)
                    h16 = spool.tile([128, bb], bf16, tag="h16", bufs=sp_bufs,
                                     name=f"h16_{hb}_{ft}")
                    nc.scalar.activation(h16, u2, AF.Gelu,
                                         bias=c3[:, ft:ft + 1], scale=1.0 / 64)
                    h8 = (h8l if ft < kt else h8h)[:, ft % kt, :]
                    hr8 = (hr8l if ft < kt else hr8h)[:, ft % kt, :]
                    nc.gpsimd.tensor_copy(h8, h16)
                    nc.vector.tensor_sub(hr8, h16, h8)
                state[half]["h8"] = (h8l, h8h)
                state[half]["hr8"] = (hr8l, hr8h)
                return
            hsb = big.tile([128, ht, bb], bf16, tag="Hm", name=f"hsb_{hb}")
            for ft in range(ht):
                wt = wtile("wm1", kt, bf16, "Wd", f"w_wm1_{hb}_{ft}",
                           w_aps["wm1"][ft])
                ps = mm_ps.tile([128, bb], f32, tag="mm", name=f"psD_{hb}_{ft}")
                mm_chain(ps, wt, fr16, kt)
                # u = rs*(P - m*c1) + c3 ; hmid = gelu(u)
                u1 = spool.tile([128, bb], f32, tag="u1", bufs=sp_bufs,
                                name=f"u1_{hb}_{ft}")
                nc.vector.scalar_tensor_tensor(
                    u1, m1, negc1[:, ft:ft + 1], ps, OP.mult, OP.add)
                u2 = spool.tile([128, bb], bf16, tag="u2", bufs=sp_bufs,
                                name=f"u2_{hb}_{ft}")
                nc.gpsimd.tensor_mul(u2, u1, rs1)
                nc.scalar.activation(hsb[:, ft, :], u2, AF.Gelu,
                                     bias=c3[:, ft:ft + 1], scale=1.0)
            state[half]["hsb"] = hsb

        def emit_E(half):
            hb = f"h{half}"
            st = state[half]
            fr16, m1, rs1 = st["fr16"], st["m1"], st["rs1"]
            for ft in range(kt):
                if mlp_dr:
                    wa = wtile("wm2a", ht, fp8, "We8", f"w_wm2a_{hb}_{ft}",
                               w_aps["wm2a"][ft])
                    wr = wtile("wm2r", ht, fp8, "We8", f"w_wm2r_{hb}_{ft}",
                               w_aps["wm2r"][ft])
                    ps = mm_ps.tile([128, bb], f32, tag="mm",
                                    name=f"psE_{hb}_{ft}")
                    dr_chain3(ps, wa, wr, list(st["h8"]), list(st["hr8"]))
                else:
                    wt = wtile("wm2", ht, bf16, "We", f"w_wm2_{hb}_{ft}",
                               w_aps["wm2"][ft])
                    ps = mm_ps.tile([128, bb], f32, tag="mm",
                                    name=f"psE_{hb}_{ft}")
                    mm_chain(ps, wt, st["hsb"], ht)
                # resid = (fr16 - m1) * g1 * rs1  (LN1 output minus bias)
                r1 = spool.tile([128, bb], f32, tag="r1", bufs=sp_bufs,
                                name=f"r1_{hb}_{ft}")
                nc.gpsimd.tensor_sub(r1, fr16[:, ft, :], m1)
                r2 = spool.tile([128, bb], f32, tag="r2", bufs=sp_bufs,
                                name=f"r2_{hb}_{ft}")
                nc.vector.scalar_tensor_tensor(
                    r2, r1, g1s[:, ft:ft + 1], rs1, OP.mult, OP.mult)
                if mlp_dr:
                    # z = P/64 + resid + (mlp_b2 + ln1_b), overwrites fr16[ft]
                    za = spool.tile([128, bb], f32, tag="za", bufs=sp_bufs,
                                    name=f"za_{hb}_{ft}")
                    nc.vector.scalar_tensor_tensor(
                        za, ps, 1.0 / 64, r2, OP.mult, OP.add)
                    nc.vector.tensor_scalar_add(
                        fr16[:, ft, :], za, bm2b1[:, ft:ft + 1])
                else:
                    # z = P + (mlp_b2 + ln1_b) + resid, overwrites fr16[ft]
                    nc.vector.scalar_tensor_tensor(
                        fr16[:, ft, :], ps, bm2b1[:, ft:ft + 1], r2,
                        OP.add, OP.add)

        def emit_LN2stats(half):
            hb = f"h{half}"
            st = state[half]
            m2, rs2 = stats(st["fr16"], "b", hb)
            state[half]["m2"] = m2
            state[half]["rs2"] = rs2

        def emit_LN2norm(half):
            hb = f"h{half}"
            b0 = half * bb
            st = state[half]
            z, m2, rs2 = st["fr16"], st["m2"], st["rs2"]
            for ft in range(kt):
                o1 = spool.tile([128, bb], f32, tag="o1", bufs=sp_bufs,
                                name=f"o1_{hb}_{ft}")
                nc.gpsimd.tensor_sub(o1, z[:, ft, :], m2)
                o2 = spool.tile([128, bb], f32, tag="o2", bufs=sp_bufs,
                                name=f"o2_{hb}_{ft}")
                nc.vector.scalar_tensor_tensor(
                    o2, o1, gln2[:, ft:ft + 1], rs2, OP.mult, OP.mult)
                ot = opool.tile([128, bb], f32, tag="o", name=f"ot_{hb}_{ft}")
                nc.scalar.activation(ot, o2, AF.Identity,
                                     bias=bln2[:, ft:ft + 1], scale=1.0)
                nc.sync.dma_start(out_ap[ft, :, b0:b0 + bb], ot)

        if reps > 1:
            loop_ctx = tc.For_i(0, reps, 1,
                                hint_engines=tuple(nc.engines.keys()))
        else:
            loop_ctx = contextlib.nullcontext()
        ctx.enter_context(loop_ctx)

        upto = tune.get("upto")
        if upto:
            # timing probe: truncate after a phase, emit one dummy output DMA
            order = ["A", "B", "D", "E"]
            fns = {"A": emit_A, "B": emit_B, "D": emit_D, "E": emit_E}
            for half in range(n_half):
                for ph in order[:order.index(upto) + 1]:
                    fns[ph](half)
            src = state[0]["fr16"] if upto != "A" else None
            ot = opool.tile([128, bb], f32, tag="o", name="probe_o")
            if src is not None:
                nc.scalar.activation(ot, src[:, 0, :], AF.Identity)
            else:
                nc.scalar.activation(ot, state[0]["y8g"][:, 0, :], AF.Identity)
            nc.sync.dma_start(out_ap[0, :, 0:bb], ot)
        else:
            for half in range(n_half):
                emit_A(half)
                if interleave and half > 0:
                    emit_LN2norm(half - 1)
                emit_B(half)
                emit_D(half)
                emit_E(half)
                emit_LN2stats(half)
                if not interleave:
                    emit_LN2norm(half)
            if interleave:
                emit_LN2norm(n_half - 1)

    return nc


# ---------------------------------------------------------------------------
# Host entry point
# ---------------------------------------------------------------------------

def _prep(inputs, d=D, h=H, b_total=B_TOTAL, n_cores=N_CORES):
    """Host-side marshalling: concat+transpose activations (split into an
    fp8 pair), ternary-quantize + pack weights, fold LN1 into mlp_w1."""
    zs = [np.asarray(inputs[k], dtype=np.float32) for k in Z_ORDER if k in inputs]
    combined = np.concatenate(zs, axis=1)  # [B, D]
    assert combined.shape == (b_total, d), combined.shape

    import concourse.mybir as _mybir

    fp8 = _mybir.dt.np(_mybir.dt.float8e4)

    xt = np.ascontiguousarray(combined.T)          # [D, B] f32
    x8 = xt.astype(fp8)
    xr8 = (xt - x8.astype(np.float32)).astype(fp8)

    tg1, sg1 = _ternary(inputs["gate_w1"])
    tt1, st1 = _ternary(inputs["tr_w1"])
    tg2, sg2 = _ternary(inputs["gate_w2"])
    tt2, st2 = _ternary(inputs["tr_w2"])

    mlp_w1 = np.asarray(inputs["mlp_w1"], dtype=np.float32)
    mlp_w2 = np.asarray(inputs["mlp_w2"], dtype=np.float32)
    g1 = np.asarray(inputs["ln1_g"], dtype=np.float32)
    b1 = np.asarray(inputs["ln1_b"], dtype=np.float32)
    w1g = mlp_w1 * g1[None, :]
    c1 = mlp_w1 @ g1
    c3 = mlp_w1 @ b1 + np.asarray(inputs["mlp_b1"], np.float32)

    SC = np.float32(64.0)
    w1s = w1g * SC
    w1a8 = w1s.astype(fp8)
    w1r8 = (w1s - w1a8.astype(np.float32)).astype(fp8)
    w2s = mlp_w2 * SC
    w2a8 = w2s.astype(fp8)
    w2r8 = (w2s - w2a8.astype(np.float32)).astype(fp8)

    shared = {
        "wm1a": _pack_lhsT(w1a8, fp8),
        "wm1r": _pack_lhsT(w1r8, fp8),
        "wm2a": _pack_lhsT(w2a8, fp8),
        "wm2r": _pack_lhsT(w2r8, fp8),
        "negc164": _pack_vec(-c1 * SC),
        "wg1": _pack_lhsT(tg1, fp8),
        "wt1": _pack_lhsT(tt1, fp8),
        "wg2": _pack_lhsT(tg2, fp8),
        "wt2": _pack_lhsT(tt2, fp8),
        "wg1s": _pack_lhsT_swi(tg1, fp8),
        "wt1s": _pack_lhsT_swi(tt1, fp8),
        "wg2s": _pack_lhsT_swi(tg2, fp8),
        "wt2s": _pack_lhsT_swi(tt2, fp8),
        "wm1": _pack_lhsT(w1g),
        "wm2": _pack_lhsT(mlp_w2),
        "bg1": _pack_vec(inputs["gate_b1"]),
        "bt1": _pack_vec(inputs["tr_b1"]),
        "btanh": _pack_vec(np.asarray(inputs["gate_b2"], np.float32) * np.float32(0.6)),
        "bthalf": _pack_vec(np.asarray(inputs["tr_b2"], np.float32) * np.float32(0.5)),
        "negc1": _pack_vec(-c1),
        "c3": _pack_vec(c3),
        "bm2b1": _pack_vec(np.asarray(inputs["mlp_b2"], np.float32) + b1),
        "g1s": _pack_vec(g1),
        "gln2": _pack_vec(inputs["ln2_g"]),
        "bln2": _pack_vec(inputs["ln2_b"]),
    }

    b_core = b_total // n_cores
    kt = d // 128
    in_maps = []
    for c in range(n_cores):
        sl = slice(c * b_core, (c + 1) * b_core)
        in_maps.append({
            "x8": np.ascontiguousarray(x8[:, sl].reshape(kt, 128, b_core)),
            "xr8": np.ascontiguousarray(xr8[:, sl].reshape(kt, 128, b_core)),
            **shared,
        })
    return in_maps, (sg1, st1, sg2, st2)


def _assemble(results, d=D, b_total=B_TOTAL, n_cores=N_CORES):
    b_core = b_total // n_cores
    out = np.empty((b_total, d), dtype=np.float32)
    for c, r in enumerate(results):
        out[c * b_core:(c + 1) * b_core] = (
            r["outT"].transpose(2, 0, 1).reshape(b_core, d))
    return out


def _make_nc(num_devices=N_CORES):
    from concourse import bacc

    return bacc.Bacc("TRN2", target_bir_lowering=False, debug=False,
                     enable_asserts=False, num_devices=num_devices)


def kernel(**inputs):
    os.environ.setdefault("BASS_NEVER_TRACE", "1")
    from concourse.bass_utils import run_bass_kernel_spmd

    in_maps, scales = _prep(inputs)
    nc = _make_nc()
    _build(nc, scales)
    nc.compile()
    res = run_bass_kernel_spmd(nc, in_maps, core_ids=list(range(N_CORES)))
    return _assemble(res.results)
